# revision 28
# baseline (speedup 1.0000x reference)
"""AssistedExcitation distributed Bass kernel for 8 TRN2 NeuronCores.

Reference computation (per batch b):
    mask[h,w]  = union over 32 boxes of axis-aligned rectangles (rasterized
                 from normalized xywh boxes, trunc + clamp semantics)
    att        = 5x5 conv of reflect-padded mask with the given kernel
    out        = x + att * x        (att broadcast over 256 channels)

Sharding: pure data parallel - batch 16 is split 2-per-core across 8 cores.

The kernel is DMA bound (ridge regime): 6.55 MB in + 6.55 MB out at fp16
per core across 16 SDMA engines.  Engine 15 (hosting the DGE queue
doorbells) only sustains ~21 GB/s vs ~25.4 for the rest, and the HWDGE
splits every DMA into EQUAL line-blocks (smallest block >= lines/16 that
divides the line count evenly) assigned to engines in order from engine 0
- so engine 15 serves the last 8 lines of every 128-line transfer and
its ~20% slowness sets the tail (v2 measured: all engines finish 45.0us,
engine 15 at 49.3us -> exec 52.3us).

v4 engine rebalance: 3 of the 16 main transfers are issued as a
[120, CH] + [8, CH] pair.  A 120-line DMA splits 15x8 (engine 15 gets
NOTHING - the even-split rule), and the 8-line remainder rides engines
0-7.  Engine 15 drops to 104 lines (~31.8us) ~= everyone else (~33us).
Line->engine assignment is per-DMA block order, NOT partition-wired, so
the tiles stay [128, CH] and the multiplies are untouched.

Other structure (from v2):
  * flat [80,80]->[1,6400] DMAs on the scalar HWDGE queue
  * PSUM->SBUF broadcast evictions split DVE {3,8} / scalar (rest),
    k-major, so multiply (b,c,k0) fires as soon as cols 0:3584 land
  * multiplies in-place; out triggers pinned on sync after the in flood,
    each gated on its producing multiply
"""

import numpy as np

import concourse.bass as bass
import concourse.tile as tile
from concourse import bacc, mybir
from concourse.tile_rust import add_dep_helper
from concourse.bass_utils import run_bass_kernel_spmd

F32 = mybir.dt.float32
F16 = mybir.dt.float16
ALU = mybir.AluOpType
ACT = mybir.ActivationFunctionType

N_CORES = 8
B, C, H, W, NBOX = 16, 256, 80, 80, 32
B_LOC = B // N_CORES          # 2 batches per core
HW = H * W                    # 6400
PAD = 84                      # 80 + 2*2 reflect pad
KS = 5
CH = 3200                     # free-dim chunk of the x stream
N_CHUNK = HW // CH            # 2
BC_CH = 512                   # psum bank width for the broadcast matmul
SPLIT_IN = {0, 4}             # chunk ids whose in-DMA is [120]+[8]
SPLIT_OUT = {7}               # chunk ids whose out-DMA is [120]+[8]
MH = CH // 2                  # multiply/out half-chunk (1600 cols)

NB2 = B_LOC * NBOX            # 64
C32_OFF = KS * W + 128        # 528
CST_COLS = C32_OFF + 2 * (2 * PAD + 4)


def _build_nc():
    nc = bacc.Bacc(None, target_bir_lowering=False)

    x_d = nc.declare_dram_parameter("x", [B_LOC, C, HW], F16, isOutput=False)
    boxes_d = nc.declare_dram_parameter("boxes", [B_LOC, NBOX, 4], F32, isOutput=False)
    nc.declare_dram_parameter("kernel", [1, 1, KS, KS], F32, isOutput=False)
    # csta: box-path constants (f32 bitcast: mapped_h | mapped_l | boxes),
    # lands first so DVE box preprocessing starts ~1us earlier.
    csta_d = nc.declare_dram_parameter("csta", [NB2, 2 * (2 * PAD + 4)], F16, isOutput=False)
    # cstb: banded conv matrices + ones row (needed later, at the conv)
    cstb_d = nc.declare_dram_parameter("cstb", [PAD, KS * W + 128], F16, isOutput=False)
    out_d = nc.declare_dram_parameter("out", [B_LOC, C, HW], F16, isOutput=True)

    with tile.TileContext(nc) as tc:
        with (
            tc.tile_pool(name="const", bufs=1) as cp,
            tc.tile_pool(name="batch", bufs=2) as bp,
            tc.tile_pool(name="attbc", bufs=2) as ap_,
            tc.tile_pool(name="xin", bufs=8) as xp,
            tc.tile_pool(name="ps_small", bufs=2, space=bass.MemorySpace.PSUM) as psm,
            tc.tile_pool(name="ps_bc", bufs=4, space=bass.MemorySpace.PSUM) as pbc,
        ):
            csta = cp.tile([NB2, 2 * (2 * PAD + 4)], F16)
            cstb = cp.tile([PAD, KS * W + 128], F16)
            trig_chain = []

            def _chain(bi):
                if trig_chain:
                    add_dep_helper(bi.ins, trig_chain[-1].ins, sync=False,
                                   reason="pin trigger order")
                trig_chain.append(bi)
                return bi

            _chain(nc.sync.dma_start(csta[:], csta_d[:]))
            _chain(nc.sync.dma_start(cstb[:], cstb_d[:]))

            kc = cstb[:, 0 : KS * W]
            ones16 = cstb[0:1, KS * W : KS * W + 128]
            c32 = csta[:].bitcast(F32)
            mapped_h = c32[:, 0:PAD]
            mapped_l = c32[:, PAD : 2 * PAD]
            bx = c32[:, 2 * PAD : 2 * PAD + 4]

            # main chunks, k-major per batch (8 x [128, 3200]: this shape
            # measured 415 GB/s ramp; [128, 6400] DMAs ramp at only ~220)
            chunks = [
                (b, cb * 128, k * CH)
                for b in range(B_LOC)
                for k in range(N_CHUNK)
                for cb in range(C // 128)
            ]
            xts = {}
            in_bis = {}
            for i, (b, c0, o0) in enumerate(chunks):
                xt = xp.tile([128, CH], F16, name=f"xt{i}", tag="xt")
                if i in SPLIT_IN:
                    in_bis[i] = _chain(nc.sync.dma_start(xt[0:120, :],
                                             x_d[b, c0 : c0 + 120, o0 : o0 + CH]))
                    _chain(nc.sync.dma_start(xt[120:128, :],
                                             x_d[b, c0 + 120 : c0 + 128, o0 : o0 + CH]))
                else:
                    in_bis[i] = _chain(nc.sync.dma_start(xt[:],
                                             x_d[b, c0 : c0 + 128, o0 : o0 + CH]))
                xts[i] = xt

            # ---- box preprocessing (DVE), both batches at once; scratch
            # packed into 2 tiles (fewer tile sems -> shorter teardown)
            sc32 = cp.tile([NB2, 8], F32)   # t1 | t2 | rowc colc | vv _
            sc16 = cp.tile([NB2, 3 * PAD], F16)  # cm | rm | rv
            t1, t2 = sc32[:, 0:2], sc32[:, 2:4]
            rowc, colc, vv = sc32[:, 4:5], sc32[:, 5:6], sc32[:, 6:7]
            cm, rm = sc16[:, 0:PAD], sc16[:, PAD : 2 * PAD]
            rv = sc16[:, 2 * PAD : 3 * PAD]
            nc.vector.scalar_tensor_tensor(
                t1, bx[:, 2:4], -0.5, bx[:, 0:2], op0=ALU.mult, op1=ALU.add
            )
            nc.vector.scalar_tensor_tensor(
                t2, bx[:, 2:4], 0.5, bx[:, 0:2], op0=ALU.mult, op1=ALU.add
            )
            nc.vector.tensor_scalar(cm, mapped_h[:], t1[:, 0:1], None, op0=ALU.is_gt)
            nc.vector.scalar_tensor_tensor(
                cm, mapped_l[:], t2[:, 0:1], cm, op0=ALU.is_le, op1=ALU.mult
            )
            nc.vector.tensor_scalar(rm, mapped_h[:], t1[:, 1:2], None, op0=ALU.is_gt)
            nc.vector.scalar_tensor_tensor(
                rm, mapped_l[:], t2[:, 1:2], rm, op0=ALU.is_le, op1=ALU.mult
            )
            nc.vector.tensor_reduce(rowc, rm[:, 2:82], axis=mybir.AxisListType.X, op=ALU.add)
            nc.vector.tensor_reduce(colc, cm[:, 2:82], axis=mybir.AxisListType.X, op=ALU.add)
            nc.vector.tensor_scalar(vv, rowc, 1.5, None, op0=ALU.is_ge)
            nc.vector.scalar_tensor_tensor(
                vv, colc, 1.5, vv, op0=ALU.is_ge, op1=ALU.mult
            )
            nc.vector.tensor_scalar(rv, rm, vv, None, op0=ALU.mult)

            # ---- rasterize + conv + flatten, both batches up front
            flats = []
            for b in range(B_LOC):
                pt_ps = psm.tile([PAD, PAD], F32, tag="pt_ps")
                nc.tensor.matmul(
                    pt_ps[:],
                    cm[b * NBOX : (b + 1) * NBOX, :],
                    rv[b * NBOX : (b + 1) * NBOX, :],
                    start=True, stop=True,
                )
                ptm = bp.tile([PAD, PAD], F16)
                nc.vector.tensor_scalar(ptm[:], pt_ps[:], 0.5, None, op0=ALU.is_ge)

                att_ps = psm.tile([H, W], F32, tag="att_ps")
                for i in range(KS):
                    nc.tensor.matmul(
                        att_ps[:],
                        ptm[:, i : i + H],
                        kc[:, i * W : (i + 1) * W],
                        start=(i == 0),
                        stop=(i == KS - 1),
                    )
                att1 = bp.tile([H, W], F16)
                nc.scalar.activation(att1[:], att_ps[:], ACT.Copy, bias=1.0)

                flat = bp.tile([1, HW], F16)
                nc.scalar.dma_start(flat[:], att1[:])
                flats.append(flat)

            # ---- broadcast + evict + multiply, interleaved per batch
            # Multiplies and out-DMAs run at half-chunk (1600 col) grain,
            # gated on eviction chunks 3/6/9/12, so out triggers start as
            # early as possible and flow at ~1us cadence.
            n_bc = (HW + BC_CH - 1) // BC_CH  # 13
            DVE_EV = {3, 8}

            def _mul_out(i):
                # full-chunk multiply + out trigger (half-grain multiplies
                # measured worse: 2x per-op overhead slowed the cadence)
                b, c0, o0 = chunks[i]
                xt = xts[i]
                nc.vector.tensor_mul(xt[:], xt[:], att_bcs[b][:, o0 : o0 + CH])
                if i in SPLIT_OUT:
                    _chain(nc.sync.dma_start(out_d[b, c0 : c0 + 120, o0 : o0 + CH],
                                             xt[0:120, :]))
                    _chain(nc.sync.dma_start(out_d[b, c0 + 120 : c0 + 128, o0 : o0 + CH],
                                             xt[120:128, :]))
                else:
                    _chain(nc.sync.dma_start(out_d[b, c0 : c0 + 128, o0 : o0 + CH],
                                             xt[:]))

            att_bcs = []
            for b in range(B_LOC):
                att_bc = ap_.tile([128, HW], F16, tag="att_bc")
                att_bcs.append(att_bc)
                i0, i1, i2, i3 = b * 4, b * 4 + 1, b * 4 + 2, b * 4 + 3
                for ci in range(n_bc):
                    off = ci * BC_CH
                    cw = min(BC_CH, HW - off)
                    bc_ps = pbc.tile([128, BC_CH], F32, tag="bc_ps")
                    nc.tensor.matmul(
                        bc_ps[:, 0:cw], ones16[:], flats[b][:, off : off + cw],
                        start=True, stop=True,
                    )
                    if ci in DVE_EV:
                        nc.vector.tensor_copy(att_bc[:, off : off + cw], bc_ps[:, 0:cw])
                    else:
                        nc.scalar.copy(att_bc[:, off : off + cw], bc_ps[:, 0:cw])
                    # k0 chunks fire after ci6 (cols 0:3584 cover 0:3200),
                    # k1 chunks after the full batch is evicted.
                    if ci == 6:
                        _mul_out(i0)
                        _mul_out(i1)
                _mul_out(i2)
                _mul_out(i3)

    if not nc.is_finalized():
        nc.finalize()
    return nc


def _host_consts(ker: np.ndarray, boxes_shard: np.ndarray):
    k = ker.reshape(KS, KS).astype(np.float32)
    cstb = np.zeros((PAD, KS * W + 128), dtype=np.float16)
    for i in range(KS):
        for j in range(KS):
            w = np.arange(W)
            cstb[w + j, i * W + w] = np.float16(k[i, j])
    cstb[0, KS * W : KS * W + 128] = np.float16(1.0)
    p = np.arange(PAD, dtype=np.float32)
    mapped = np.minimum(np.abs(p - 2.0), 158.0 - (p - 2.0)).astype(np.float32)
    c32 = np.zeros((NB2, 2 * PAD + 4), dtype=np.float32)
    c32[:, 0:PAD] = ((mapped + 1.0) / np.float32(W))[None, :]
    c32[:, PAD : 2 * PAD] = (mapped / np.float32(W))[None, :]
    c32[:, 2 * PAD : 2 * PAD + 4] = boxes_shard.reshape(NB2, 4)
    return c32.view(np.float16), cstb


_NC_CACHE = None


def _get_nc():
    global _NC_CACHE
    if _NC_CACHE is None:
        _NC_CACHE = _build_nc()
    return _NC_CACHE


def _run(inputs, trace=False, **kw):
    x = np.ascontiguousarray(np.asarray(inputs["x"], dtype=np.float32))
    boxes = np.ascontiguousarray(np.asarray(inputs["boxes"], dtype=np.float32))
    ker = np.ascontiguousarray(np.asarray(inputs["kernel"], dtype=np.float32))
    assert x.shape == (B, C, H, W) and boxes.shape == (B, NBOX, 4)

    nc = _get_nc()
    x16 = x.astype(np.float16).reshape(B, C, HW)
    in_maps = []
    for i in range(N_CORES):
        bsh = boxes[i * B_LOC : (i + 1) * B_LOC]
        csta, cstb = _host_consts(ker, bsh)
        in_maps.append(
            {
                "x": np.ascontiguousarray(x16[i * B_LOC : (i + 1) * B_LOC]),
                "boxes": bsh,
                "kernel": ker,
                "csta": np.ascontiguousarray(csta),
                "cstb": cstb,
            }
        )
    res = run_bass_kernel_spmd(nc, in_maps, core_ids=list(range(N_CORES)),
                               trace=trace, **kw)
    out = np.concatenate([r["out"] for r in res.results], axis=0)
    return out.reshape(B, C, H, W).astype(np.float32), res


def kernel(**inputs) -> np.ndarray:
    out, _ = _run(inputs, trace=False)
    return out


# revision 30
# speedup vs baseline: 1.0172x; 1.0172x over previous
"""AssistedExcitation distributed Bass kernel for 8 TRN2 NeuronCores.

Reference computation (per batch b):
    mask[h,w]  = union over 32 boxes of axis-aligned rectangles (rasterized
                 from normalized xywh boxes, trunc + clamp semantics)
    att        = 5x5 conv of reflect-padded mask with the given kernel
    out        = x + att * x        (att broadcast over 256 channels)

Sharding: pure data parallel - batch 16 is split 2-per-core across 8 cores.

The kernel is DMA bound (ridge regime): 6.55 MB in + 6.55 MB out at fp16
per core across 16 SDMA engines.  Engine 15 (hosting the DGE queue
doorbells) only sustains ~21 GB/s vs ~25.4 for the rest, and the HWDGE
splits every DMA into EQUAL line-blocks (smallest block >= lines/16 that
divides the line count evenly) assigned to engines in order from engine 0
- so engine 15 serves the last 8 lines of every 128-line transfer and
its ~20% slowness sets the tail (v2 measured: all engines finish 45.0us,
engine 15 at 49.3us -> exec 52.3us).

Engine rebalance: 3 of the 16 main transfers are issued as a
[120, CH] + [8, CH] pair.  A 120-line DMA splits 15x8 (engine 15 gets
NOTHING - the even-split rule), and the 8-line remainder rides engines
0-7.  Engine 15 drops to 104 lines (~32us busy, ends ~43us) ~= the rest
(~33us, end ~47).  Line->engine assignment is per-DMA block order, NOT
partition-wired, so the tiles stay [128, CH] and multiplies untouched.

Other structure:
  * the const tensor is split: csta (boxes + reflect-mapped coordinate
    rows) lands ~9us so DVE box preprocessing starts ~1us earlier; cstb
    (banded conv matrices) lands by the conv (~11us)
  * flat [80,80]->[1,6400] DMAs ride the scalar HWDGE queue.  Measured:
    an SB->SB DMA has ~6.5-7us FIXED trigger->observable-sem latency
    (~3us doorbell/dispatch + execute + ~2us completion propagation),
    load-independent - draining the fabric first (a pinned "lull") only
    idles the flood and regresses (55-61us, measured twice)
  * PSUM->SBUF broadcast evictions split DVE {3,8} / scalar (rest),
    k-major chunk order, so multiply (b,c,k0) fires as soon as att cols
    0:3584 land; multiplies in-place at full-chunk grain (half-grain
    regressed: 2x op overhead + sync descgen saturation with 20 out
    triggers)
  * out triggers pinned on sync behind the in flood, each gated on its
    producing multiply; first fires ~25us, overlapping the in-flood tail

Measured failures kept for the record: gpsimd tensor_tensor is ~5.7
ns/elem AND knocks concurrent DVE tensor_tensor out of 2x-1p mode
(1.53->3.0us); [128, 6400] in-DMAs ramp at ~220 GB/s vs 415 for
[128, 3200]; single_packet=True on the multi-line flat hard-crashes
(NRT_EXEC_UNIT_UNRECOVERABLE); 126-line mains exclude BOTH engines
14+15 (even-split picks 14x9).
"""

import numpy as np

import concourse.bass as bass
import concourse.tile as tile
from concourse import bacc, mybir
from concourse.tile_rust import add_dep_helper
from concourse.bass_utils import run_bass_kernel_spmd

F32 = mybir.dt.float32
F16 = mybir.dt.float16
ALU = mybir.AluOpType
ACT = mybir.ActivationFunctionType

N_CORES = 8
B, C, H, W, NBOX = 16, 256, 80, 80, 32
B_LOC = B // N_CORES          # 2 batches per core
HW = H * W                    # 6400
PAD = 84                      # 80 + 2*2 reflect pad
KS = 5
CH = 3200                     # free-dim chunk of the x stream
N_CHUNK = HW // CH            # 2
BC_CH = 512                   # psum bank width for the broadcast matmul
SPLIT_IN = {0, 4}             # chunk ids whose in-DMA is [120]+[8]
SPLIT_OUT = {7}               # chunk ids whose out-DMA is [120]+[8]
MH = CH // 2                  # multiply/out half-chunk (1600 cols)

NB2 = B_LOC * NBOX            # 64
C32_OFF = KS * W + 128        # 528
CST_COLS = C32_OFF + 2 * (2 * PAD + 4)


def _build_nc():
    nc = bacc.Bacc(None, target_bir_lowering=False)

    x_d = nc.declare_dram_parameter("x", [B_LOC, C, HW], F16, isOutput=False)
    boxes_d = nc.declare_dram_parameter("boxes", [B_LOC, NBOX, 4], F32, isOutput=False)
    nc.declare_dram_parameter("kernel", [1, 1, KS, KS], F32, isOutput=False)
    # csta: box-path constants (f32 bitcast: mapped_h | mapped_l | boxes),
    # lands first so DVE box preprocessing starts ~1us earlier.
    csta_d = nc.declare_dram_parameter("csta", [NB2, 2 * (2 * PAD + 4)], F16, isOutput=False)
    # cstb: banded conv matrices + ones row (needed later, at the conv)
    cstb_d = nc.declare_dram_parameter("cstb", [PAD, KS * W + 128], F16, isOutput=False)
    out_d = nc.declare_dram_parameter("out", [B_LOC, C, HW], F16, isOutput=True)

    with tile.TileContext(nc) as tc:
        with (
            tc.tile_pool(name="const", bufs=1) as cp,
            tc.tile_pool(name="batch", bufs=2) as bp,
            tc.tile_pool(name="attbc", bufs=2) as ap_,
            tc.tile_pool(name="xin", bufs=8) as xp,
            tc.tile_pool(name="ps_small", bufs=2, space=bass.MemorySpace.PSUM) as psm,
            tc.tile_pool(name="ps_bc", bufs=4, space=bass.MemorySpace.PSUM) as pbc,
        ):
            csta = cp.tile([NB2, 2 * (2 * PAD + 4)], F16)
            cstb = cp.tile([PAD, KS * W + 128], F16)
            trig_chain = []

            def _chain(bi):
                if trig_chain:
                    add_dep_helper(bi.ins, trig_chain[-1].ins, sync=False,
                                   reason="pin trigger order")
                trig_chain.append(bi)
                return bi

            _chain(nc.sync.dma_start(csta[:], csta_d[:]))
            _chain(nc.sync.dma_start(cstb[:], cstb_d[:]))

            kc = cstb[:, 0 : KS * W]
            ones16 = cstb[0:1, KS * W : KS * W + 128]
            c32 = csta[:].bitcast(F32)
            mapped_h = c32[:, 0:PAD]
            mapped_l = c32[:, PAD : 2 * PAD]
            bx = c32[:, 2 * PAD : 2 * PAD + 4]

            # main chunks, k-major per batch (8 x [128, 3200]: this shape
            # measured 415 GB/s ramp; [128, 6400] DMAs ramp at only ~220)
            chunks = [
                (b, cb * 128, k * CH)
                for b in range(B_LOC)
                for k in range(N_CHUNK)
                for cb in range(C // 128)
            ]
            xts = {}
            in_bis = {}
            for i, (b, c0, o0) in enumerate(chunks):
                xt = xp.tile([128, CH], F16, name=f"xt{i}", tag="xt")
                if i in SPLIT_IN:
                    in_bis[i] = _chain(nc.sync.dma_start(xt[0:120, :],
                                             x_d[b, c0 : c0 + 120, o0 : o0 + CH]))
                    _chain(nc.sync.dma_start(xt[120:128, :],
                                             x_d[b, c0 + 120 : c0 + 128, o0 : o0 + CH]))
                else:
                    in_bis[i] = _chain(nc.sync.dma_start(xt[:],
                                             x_d[b, c0 : c0 + 128, o0 : o0 + CH]))
                xts[i] = xt

            # ---- box preprocessing (DVE), both batches at once
            t1 = cp.tile([NB2, 2], F32)
            nc.vector.scalar_tensor_tensor(
                t1[:], bx[:, 2:4], -0.5, bx[:, 0:2], op0=ALU.mult, op1=ALU.add
            )
            t2 = cp.tile([NB2, 2], F32)
            nc.vector.scalar_tensor_tensor(
                t2[:], bx[:, 2:4], 0.5, bx[:, 0:2], op0=ALU.mult, op1=ALU.add
            )
            cm = cp.tile([NB2, PAD], F16)
            nc.vector.tensor_scalar(cm[:], mapped_h[:], t1[:, 0:1], None, op0=ALU.is_gt)
            nc.vector.scalar_tensor_tensor(
                cm[:], mapped_l[:], t2[:, 0:1], cm[:], op0=ALU.is_le, op1=ALU.mult
            )
            rm = cp.tile([NB2, PAD], F16)
            nc.vector.tensor_scalar(rm[:], mapped_h[:], t1[:, 1:2], None, op0=ALU.is_gt)
            nc.vector.scalar_tensor_tensor(
                rm[:], mapped_l[:], t2[:, 1:2], rm[:], op0=ALU.is_le, op1=ALU.mult
            )
            rowc = cp.tile([NB2, 1], F32)
            nc.vector.tensor_reduce(rowc[:], rm[:, 2:82], axis=mybir.AxisListType.X, op=ALU.add)
            colc = cp.tile([NB2, 1], F32)
            nc.vector.tensor_reduce(colc[:], cm[:, 2:82], axis=mybir.AxisListType.X, op=ALU.add)
            vv = cp.tile([NB2, 1], F32)
            nc.vector.tensor_scalar(vv[:], rowc[:], 1.5, None, op0=ALU.is_ge)
            nc.vector.scalar_tensor_tensor(
                vv[:], colc[:], 1.5, vv[:], op0=ALU.is_ge, op1=ALU.mult
            )
            rv = cp.tile([NB2, PAD], F16)
            nc.vector.tensor_scalar(rv[:], rm[:], vv[:], None, op0=ALU.mult)

            # ---- rasterize + conv + flatten, both batches up front
            flats = []
            for b in range(B_LOC):
                pt_ps = psm.tile([PAD, PAD], F32, tag="pt_ps")
                nc.tensor.matmul(
                    pt_ps[:],
                    cm[b * NBOX : (b + 1) * NBOX, :],
                    rv[b * NBOX : (b + 1) * NBOX, :],
                    start=True, stop=True,
                )
                ptm = bp.tile([PAD, PAD], F16)
                nc.vector.tensor_scalar(ptm[:], pt_ps[:], 0.5, None, op0=ALU.is_ge)

                att_ps = psm.tile([H, W], F32, tag="att_ps")
                for i in range(KS):
                    nc.tensor.matmul(
                        att_ps[:],
                        ptm[:, i : i + H],
                        kc[:, i * W : (i + 1) * W],
                        start=(i == 0),
                        stop=(i == KS - 1),
                    )
                att1 = bp.tile([H, W], F16)
                nc.scalar.activation(att1[:], att_ps[:], ACT.Copy, bias=1.0)

                flat = bp.tile([1, HW], F16)
                nc.scalar.dma_start(flat[:], att1[:])
                flats.append(flat)

            # ---- broadcast + evict + multiply, interleaved per batch
            # Multiplies and out-DMAs run at half-chunk (1600 col) grain,
            # gated on eviction chunks 3/6/9/12, so out triggers start as
            # early as possible and flow at ~1us cadence.
            n_bc = (HW + BC_CH - 1) // BC_CH  # 13
            DVE_EV = {3, 8}

            def _mul_out(i):
                # full-chunk multiply + out trigger (half-grain multiplies
                # measured worse: 2x per-op overhead slowed the cadence)
                b, c0, o0 = chunks[i]
                xt = xts[i]
                nc.vector.tensor_mul(xt[:], xt[:], att_bcs[b][:, o0 : o0 + CH])
                if i in SPLIT_OUT:
                    _chain(nc.sync.dma_start(out_d[b, c0 : c0 + 120, o0 : o0 + CH],
                                             xt[0:120, :]))
                    _chain(nc.sync.dma_start(out_d[b, c0 + 120 : c0 + 128, o0 : o0 + CH],
                                             xt[120:128, :]))
                else:
                    _chain(nc.sync.dma_start(out_d[b, c0 : c0 + 128, o0 : o0 + CH],
                                             xt[:]))

            att_bcs = []
            for b in range(B_LOC):
                att_bc = ap_.tile([128, HW], F16, tag="att_bc")
                att_bcs.append(att_bc)
                i0, i1, i2, i3 = b * 4, b * 4 + 1, b * 4 + 2, b * 4 + 3
                for ci in range(n_bc):
                    off = ci * BC_CH
                    cw = min(BC_CH, HW - off)
                    bc_ps = pbc.tile([128, BC_CH], F32, tag="bc_ps")
                    nc.tensor.matmul(
                        bc_ps[:, 0:cw], ones16[:], flats[b][:, off : off + cw],
                        start=True, stop=True,
                    )
                    if ci in DVE_EV:
                        nc.vector.tensor_copy(att_bc[:, off : off + cw], bc_ps[:, 0:cw])
                    else:
                        nc.scalar.copy(att_bc[:, off : off + cw], bc_ps[:, 0:cw])
                    # k0 chunks fire after ci6 (cols 0:3584 cover 0:3200),
                    # k1 chunks after the full batch is evicted.
                    if ci == 6:
                        _mul_out(i0)
                        _mul_out(i1)
                _mul_out(i2)
                _mul_out(i3)

    if not nc.is_finalized():
        nc.finalize()
    return nc


def _host_consts(ker: np.ndarray, boxes_shard: np.ndarray):
    k = ker.reshape(KS, KS).astype(np.float32)
    cstb = np.zeros((PAD, KS * W + 128), dtype=np.float16)
    for i in range(KS):
        for j in range(KS):
            w = np.arange(W)
            cstb[w + j, i * W + w] = np.float16(k[i, j])
    cstb[0, KS * W : KS * W + 128] = np.float16(1.0)
    p = np.arange(PAD, dtype=np.float32)
    mapped = np.minimum(np.abs(p - 2.0), 158.0 - (p - 2.0)).astype(np.float32)
    c32 = np.zeros((NB2, 2 * PAD + 4), dtype=np.float32)
    c32[:, 0:PAD] = ((mapped + 1.0) / np.float32(W))[None, :]
    c32[:, PAD : 2 * PAD] = (mapped / np.float32(W))[None, :]
    c32[:, 2 * PAD : 2 * PAD + 4] = boxes_shard.reshape(NB2, 4)
    return c32.view(np.float16), cstb


_NC_CACHE = None


def _get_nc():
    global _NC_CACHE
    if _NC_CACHE is None:
        _NC_CACHE = _build_nc()
    return _NC_CACHE


def _run(inputs, trace=False, **kw):
    x = np.ascontiguousarray(np.asarray(inputs["x"], dtype=np.float32))
    boxes = np.ascontiguousarray(np.asarray(inputs["boxes"], dtype=np.float32))
    ker = np.ascontiguousarray(np.asarray(inputs["kernel"], dtype=np.float32))
    assert x.shape == (B, C, H, W) and boxes.shape == (B, NBOX, 4)

    nc = _get_nc()
    x16 = x.astype(np.float16).reshape(B, C, HW)
    in_maps = []
    for i in range(N_CORES):
        bsh = boxes[i * B_LOC : (i + 1) * B_LOC]
        csta, cstb = _host_consts(ker, bsh)
        in_maps.append(
            {
                "x": np.ascontiguousarray(x16[i * B_LOC : (i + 1) * B_LOC]),
                "boxes": bsh,
                "kernel": ker,
                "csta": np.ascontiguousarray(csta),
                "cstb": cstb,
            }
        )
    res = run_bass_kernel_spmd(nc, in_maps, core_ids=list(range(N_CORES)),
                               trace=trace, **kw)
    out = np.concatenate([r["out"] for r in res.results], axis=0)
    return out.reshape(B, C, H, W).astype(np.float32), res


def kernel(**inputs) -> np.ndarray:
    out, _ = _run(inputs, trace=False)
    return out
